# revision 1
# baseline (speedup 1.0000x reference)
"""Trainium2 Bass kernel for Luong local-p sparse attention.

Math (per batch n, full shapes N=64, L=258, H=1024, Q=256):
    score = (h_t @ W_a) @ enc^T           masked to window [p_t-16, p_t+16]
    align = softmax(score) * gauss(p_t)
    out   = tanh([align @ enc, h_t] @ W_c^T)

Only a 33-wide window of enc survives the mask (window is +-16 around p_t),
so the kernel gathers windows host-side and pushes W_a / W_c[:, :H] through
the 33-wide side:
    u  = W_a-transform of window   (uT[h, (n,j)]  = sum_k W_aT[k,h] enc_w[(n,j),k])
    s  = uT^T-partial scores       (score^T[j, q] = sum_h uT[h,j] h_t[q,h])
    softmax over j (33 rows) done j-major with a 4th-power renormalization
    trick (no partition-dim max needed; partition sums via ones-matmul)
    v  = W_c1-transform of window  (v[(n,j), h']  = sum_h enc_w[(n,j),h] W_c1T[h,h'])
    out = tanh(h_t @ W_c2T + align^T.T @ v)

Data parallel over batch: 8 batches per core x 8 cores.  All matmuls run as
float32r (full-rate fp32 streaming mode).
"""

import numpy as np

import concourse.bass as bass
import concourse.bacc as bacc
import concourse.mybir as mybir
import concourse.tile as tile
from concourse.bass_utils import run_bass_kernel_spmd

# Problem constants (hardcoded per harness contract).
N, L, H, Q = 64, 258, 1024, 256
WINDOW = 16.0
DEV_POW = 128.0
NCORES = 8
B = N // NCORES  # batches per core
W = 33           # window width (positions that can survive the mask)
HC = H // 128    # h-chunks of 128 (PE contraction tiles)
F32 = mybir.dt.float32
F32R = mybir.dt.float32r
AF = mybir.ActivationFunctionType

# exp is computed as t = exp(s/4 + bias); bias = LOG_ALPHA keeps the
# column-sum T = sum_j t below fp32 max (t <= e^83, T <= 33*e^83 < e^88.7).
# The alpha scale cancels exactly in w = t/T.
LOG_ALPHA = -4.8520302  # -7*ln(2)
MASK_BIAS = -10000.0    # exp(<= -9900) == 0 in fp32


def build_nc() -> bass.Bass:
    nc = bacc.Bacc()
    enc_wT = nc.declare_dram_parameter("enc_wT", [H, B * W], F32R, isOutput=False)
    dec_hT = nc.declare_dram_parameter("dec_hT", [H, B * Q], F32R, isOutput=False)
    W_aT = nc.declare_dram_parameter("W_aT", [H, H], F32R, isOutput=False)
    W_c1T = nc.declare_dram_parameter("W_c1T", [H, H], F32R, isOutput=False)
    W_c2T = nc.declare_dram_parameter("W_c2T", [H, H], F32R, isOutput=False)
    biasT = nc.declare_dram_parameter("biasT", [W, B], F32, isOutput=False)
    gT = nc.declare_dram_parameter("gT", [W, B], F32, isOutput=False)
    onesD = nc.declare_dram_parameter("onesD", [W, W], F32R, isOutput=False)
    gPackT = nc.declare_dram_parameter("gPackT", [3 * W, 3], F32, isOutput=False)
    out = nc.declare_dram_parameter("out", [B * Q, H], F32, isOutput=True)

    with tile.TileContext(nc) as tc:
        with (
            tc.tile_pool(name="const", bufs=1) as cpool,
            tc.tile_pool(name="wc1p", bufs=8) as wc1p,
            tc.tile_pool(name="vstp", bufs=3) as vstp,
            tc.tile_pool(name="dec", bufs=2) as dec_pool,
            tc.tile_pool(name="sm", bufs=2) as sm_pool,
            tc.tile_pool(name="outp", bufs=2) as out_pool,
            tc.tile_pool(name="psA", bufs=2, space="PSUM") as psA,
            tc.tile_pool(name="psB", bufs=6, space="PSUM") as psB,
        ):
            # ---------------- resident tensors ----------------
            enc_sb = cpool.tile([128, HC, B * W], F32R)
            WaT_sb = cpool.tile([128, HC, H], F32R)
            Wc2T_sb = cpool.tile([128, HC, H], F32R)
            bias_sb = cpool.tile([W, B], F32)
            g_sb = cpool.tile([W, B], F32)
            gpack_sb = cpool.tile([3 * W, 3], F32)
            ones_sb = cpool.tile([W, W], F32R)
            uT_sb = cpool.tile([128, HC, B * W], F32R)
            v_sb = cpool.tile([W, B, H], F32R)

            enc_r = enc_wT[:, :].rearrange("(c p) m -> p c m", p=128)
            WaT_r = W_aT[:, :].rearrange("(c p) m -> p c m", p=128)
            Wc2_r = W_c2T[:, :].rearrange("(c p) m -> p c m", p=128)
            Wc1_r = W_c1T[:, :].rearrange("(cp p) m -> p cp m", p=128)
            dec_r = dec_hT[:, :].rearrange("(c p) (n q) -> p c n q", p=128, q=Q)

            # DMA issue order is the schedule.  sync ring is ~1.5x faster:
            # it carries enc + most of W_aT + all of W_c2T; scalar carries
            # the rest of W_aT + most of W_c1T + early dec batches.
            nc.sync.dma_start(out=enc_sb, in_=enc_r)
            nc.sync.dma_start(out=WaT_sb[:, 0:2, :], in_=WaT_r[:, 0:2, :])
            nc.scalar.dma_start(out=WaT_sb[:, 4:6, :], in_=WaT_r[:, 4:6, :])
            nc.sync.dma_start(out=WaT_sb[:, 2:4, :], in_=WaT_r[:, 2:4, :])
            nc.scalar.dma_start(out=WaT_sb[:, 6:8, :], in_=WaT_r[:, 6:8, :])
            nc.scalar.dma_start(out=bias_sb, in_=biasT[:, :])
            nc.scalar.dma_start(out=g_sb, in_=gT[:, :])
            nc.scalar.dma_start(out=gpack_sb, in_=gPackT[:, :])
            nc.scalar.dma_start(out=ones_sb, in_=onesD[:, :])

            dec_tiles = {}
            for n in range(2):
                dt_ = dec_pool.tile([128, HC, Q], F32R, tag="dec", name=f"dec{n}")
                eng = nc.sync if n % 2 == 0 else nc.scalar
                eng.dma_start(out=dt_, in_=dec_r[:, :, n, :])
                dec_tiles[n] = dt_

            # W_c2T on the fast ring (batch 0's dec_out is paced by it).
            for i in range(2):
                nc.sync.dma_start(out=Wc2T_sb[:, 4 * i:4 * i + 4, :], in_=Wc2_r[:, 4 * i:4 * i + 4, :])

            # All W_c1T chunks resident (consumed by the v phase), mostly on
            # the scalar ring.
            wc1_tiles = {}
            for nt in range(2):
                for kcp in range(4):
                    wt = wc1p.tile([128, 2, 512], F32R, tag="wc1", name=f"wc1_{nt}_{kcp}")
                    eng = nc.sync if kcp == 3 else nc.scalar
                    eng.dma_start(
                        out=wt,
                        in_=Wc1_r[:, 2 * kcp:2 * kcp + 2, nt * 512:(nt + 1) * 512],
                    )
                    wc1_tiles[(nt, kcp)] = wt

            # ---------------- u phase: uT[h, (n,j)] ----------------
            for hc in range(HC):
                pu = psB.tile([128, B * W], F32, tag="B", name=f"pu{hc}")
                for kc in range(HC):
                    nc.tensor.matmul(
                        pu,
                        lhsT=WaT_sb[:, kc, hc * 128:(hc + 1) * 128],
                        rhs=enc_sb[:, kc, :],
                        start=(kc == 0),
                        stop=(kc == HC - 1),
                    )
                nc.scalar.copy(out=uT_sb[:, hc, :], in_=pu)

            # ---------------- v phase helpers (emitted between batches) ----
            GROUPS = [(0, 99), (99, 99), (198, 66)]

            def v_group(gi):
                g0, glen = GROUPS[gi]
                for nt in range(2):
                    pv = psB.tile([128, 512], F32, tag="B", name=f"pv{nt}_{gi}")
                    for kcp in range(4):
                        for j in range(2):
                            kc = 2 * kcp + j
                            nc.tensor.matmul(
                                pv[:glen, :],
                                lhsT=enc_sb[:, kc, g0:g0 + glen],
                                rhs=wc1_tiles[(nt, kcp)][:, j, :],
                                start=(kc == 0),
                                stop=(kc == HC - 1),
                            )
                    vst = vstp.tile([128, 512], F32R, tag="vst", name=f"vst{nt}_{gi}")
                    # evacuate + fold the gaussian in one op: ctx = p4 @ (g*v)
                    nc.vector.tensor_scalar_mul(
                        vst[:glen, :], pv[:glen, :], gpack_sb[:glen, gi:gi + 1]
                    )
                    for off in range(glen // W):
                        n = gi * 3 + off
                        eng = nc.sync if (off + nt) % 2 == 0 else nc.scalar
                        eng.dma_start(
                            out=v_sb[:, n, nt * 512:(nt + 1) * 512],
                            in_=vst[off * W:(off + 1) * W, :],
                        )

            # ---------------- per-batch emission ----------------
            prev = None  # (n, pos, o_sb) awaiting tanh + store

            def flush_prev(split_store=False):
                nonlocal prev
                if prev is None:
                    return
                pn, ppos, po_sb = prev
                for qt in range(2):
                    for ht in range(2):
                        nc.scalar.activation(
                            out=po_sb[:, qt, ht * 512:(ht + 1) * 512],
                            in_=ppos[(qt, ht)], func=AF.Tanh,
                        )
                dst = out[pn * Q:(pn + 1) * Q, :].rearrange("(qt p) h -> p qt h", p=128)
                if split_store:
                    nc.sync.dma_start(out=dst[:, 0, :], in_=po_sb[:, 0, :])
                    nc.scalar.dma_start(out=dst[:, 1, :], in_=po_sb[:, 1, :])
                else:
                    eng = nc.sync if pn % 2 == 0 else nc.scalar
                    eng.dma_start(out=dst, in_=po_sb)
                prev = None

            state = {}
            scored = {}

            def score_part(n):
                if n in dec_tiles:
                    dec_sb = dec_tiles[n]
                else:
                    dec_sb = dec_pool.tile([128, HC, Q], F32R, tag="dec", name=f"dec{n}")
                    eng = nc.sync if n % 2 == 0 else nc.scalar
                    eng.dma_start(out=dec_sb, in_=dec_r[:, :, n, :])

                ps = psA.tile([W, Q], F32, tag="A", name=f"ps{n}")
                for hc in range(HC):
                    nc.tensor.matmul(
                        ps,
                        lhsT=uT_sb[:, hc, n * W:(n + 1) * W],
                        rhs=dec_sb[:, hc, :],
                        start=(hc == 0),
                        stop=(hc == HC - 1),
                    )
                # softmax over 33 partitions via 4th-power renormalization,
                # in place in one tile: t=exp(s/4+b); T=colsum t; t=(t/T)^4;
                # Z=colsum t; t/=Z (-> p4).  PE bits hide in the dec stream.
                t = sm_pool.tile([W, Q], F32R, tag="t", name=f"t{n}")
                nc.scalar.activation(
                    out=t, in_=ps, func=AF.Exp, bias=bias_sb[:, n:n + 1], scale=0.25
                )
                scored[n] = (dec_sb, t)

            def batch_pre(n):
                if n not in scored:
                    score_part(n)
                dec_sb, t = scored.pop(n)
                flush_prev()
                o_sb = out_pool.tile([128, 2, H], F32, tag="o", name=f"o{n}")
                pos = {}

                def dec_group(qt, ht):
                    po = psB.tile([128, 512], F32, tag="B", name=f"po{n}_{qt}_{ht}")
                    pos[(qt, ht)] = po
                    for hc in range(HC):
                        nc.tensor.matmul(
                            po,
                            lhsT=dec_sb[:, hc, qt * 128:(qt + 1) * 128],
                            rhs=Wc2T_sb[:, hc, ht * 512:(ht + 1) * 512],
                            start=(hc == 0),
                            stop=False,
                        )

                dec_group(0, 0)
                pT = psA.tile([W, Q], F32, tag="A", name=f"pT{n}")
                nc.tensor.matmul(pT, lhsT=ones_sb[:], rhs=t[:], start=True, stop=True)
                rT = sm_pool.tile([W, Q], F32, tag="rT", name=f"rT{n}")
                nc.vector.reciprocal_approx_fast(out=rT, in_=pT)
                nc.vector.tensor_mul(t, t, rT)
                nc.vector.tensor_mul(t, t, t)
                nc.vector.tensor_mul(t, t, t)
                dec_group(0, 1)
                dec_group(1, 0)
                pZ = psA.tile([W, Q], F32, tag="A", name=f"pZ{n}")
                nc.tensor.matmul(pZ, lhsT=ones_sb[:], rhs=t[:], start=True, stop=True)
                rZ = sm_pool.tile([W, Q], F32, tag="rZ", name=f"rZ{n}")
                nc.vector.reciprocal_approx_fast(out=rZ, in_=pZ)
                nc.vector.tensor_mul(t, t, rZ)
                dec_group(1, 1)
                state[n] = (t, pos, o_sb)

            def batch_ctx(n):
                t, pos, o_sb = state.pop(n)
                last = n == B - 1
                dst = out[n * Q:(n + 1) * Q, :].rearrange("(qt p) h -> p qt h", p=128)
                for qt in range(2):
                    for ht in range(2):
                        nc.tensor.matmul(
                            pos[(qt, ht)],
                            lhsT=t[:, qt * 128:(qt + 1) * 128],
                            rhs=v_sb[:, n, ht * 512:(ht + 1) * 512],
                            start=False,
                            stop=True,
                        )
                        if last:
                            # drain the tail: tanh + store as soon as each
                            # group closes instead of pipelining into a
                            # (nonexistent) next batch
                            nc.scalar.activation(
                                out=o_sb[:, qt, ht * 512:(ht + 1) * 512],
                                in_=pos[(qt, ht)], func=AF.Tanh,
                            )
                    if last:
                        eng = nc.sync if qt == 0 else nc.scalar
                        eng.dma_start(out=dst[:, qt, :], in_=o_sb[:, qt, :])
                nonlocal prev
                if not last:
                    prev = (n, pos, o_sb)

            batch_pre(0)
            v_group(0)
            batch_ctx(0)
            batch_pre(1)
            batch_ctx(1)
            batch_pre(2)
            batch_ctx(2)
            v_group(1)
            batch_pre(3)
            batch_ctx(3)
            batch_pre(4)
            batch_ctx(4)
            v_group(2)
            for n in range(5, B):
                batch_pre(n)
                batch_ctx(n)
            flush_prev(split_store=True)
    nc.compile()
    return nc


def round_f32r(a: np.ndarray) -> np.ndarray:
    """Round fp32 to fp32r (TF32-like: 11-bit mantissa, low 12 bits zero),
    round-to-nearest-even.  This is what the PE consumes in fp32r mode."""
    u = np.ascontiguousarray(a, dtype=np.float32).view(np.uint32)
    lsb = (u >> np.uint32(12)) & np.uint32(1)
    u = (u + np.uint32(0x7FF) + lsb) & np.uint32(0xFFFFF000)
    return u.view(np.float32)


def prepare_in_maps(inputs: dict) -> list[dict]:
    enc = np.asarray(inputs["encoder_outputs"], dtype=np.float32)
    dec = np.asarray(inputs["decoder_h_t"], dtype=np.float32)
    src_len = np.asarray(inputs["src_len"], dtype=np.int32)
    p_t = np.asarray(inputs["p_t"], dtype=np.float32)
    W_a = np.asarray(inputs["W_a"], dtype=np.float32)
    W_c = np.asarray(inputs["W_c"], dtype=np.float32)

    # Window bounds, computed with the same fp32 ops as the reference.
    attn_start = np.maximum(p_t - np.float32(WINDOW), np.float32(0.0))
    attn_end = np.minimum(p_t + np.float32(WINDOW), src_len.astype(np.float32))
    s = np.ceil(attn_start).astype(np.int64)
    s = np.minimum(s, L - W)  # keep the 33-slice in bounds
    idx = s[:, None] + np.arange(W)[None, :]
    idxf = idx.astype(np.float32)
    mask = (idxf < attn_start[:, None]) | (idxf > attn_end[:, None])
    bias = np.where(mask, np.float32(MASK_BIAS), np.float32(LOG_ALPHA)).astype(np.float32)
    g = np.exp(-((idxf - p_t[:, None]) ** 2) / np.float32(DEV_POW)).astype(np.float32)

    enc_w = round_f32r(enc[np.arange(N)[:, None], idx, :])  # [N, W, H]
    dec = round_f32r(dec)
    W_aT = round_f32r(W_a.T)
    W_c1T = round_f32r(W_c[:, :H].T)
    W_c2T = round_f32r(W_c[:, H:].T)

    in_maps = []
    for c in range(NCORES):
        bs = slice(c * B, (c + 1) * B)
        gc = g[bs]  # [B, W]
        gpack = np.zeros((3 * W, 3), dtype=np.float32)
        for n in range(B):
            gi, off = divmod(n, 3)
            gpack[off * W:(off + 1) * W, gi] = gc[n]
        in_maps.append({
            "enc_wT": np.ascontiguousarray(enc_w[bs].transpose(2, 0, 1).reshape(H, B * W)),
            "dec_hT": np.ascontiguousarray(dec[bs].transpose(2, 0, 1).reshape(H, B * Q)),
            "W_aT": W_aT,
            "W_c1T": W_c1T,
            "W_c2T": W_c2T,
            "biasT": np.ascontiguousarray(bias[bs].T),
            "onesD": np.ones((W, W), dtype=np.float32),
            "gPackT": gpack,
            "gT": np.ascontiguousarray(g[bs].T),
        })
    return in_maps


_NC = None


def get_nc() -> bass.Bass:
    global _NC
    if _NC is None:
        _NC = build_nc()
    return _NC


def kernel(**inputs) -> np.ndarray:
    nc = get_nc()
    in_maps = prepare_in_maps(inputs)
    res = run_bass_kernel_spmd(nc, in_maps, list(range(NCORES)))
    outs = [res.results[c]["out"].reshape(B, Q, H) for c in range(NCORES)]
    return np.concatenate(outs, axis=0)



# revision 12
# speedup vs baseline: 1.0143x; 1.0143x over previous
"""Trainium2 Bass kernel for Luong local-p sparse attention (fp16 edition).

Math (per batch n, full shapes N=64, L=258, H=1024, Q=256):
    score = (h_t @ W_a) @ enc^T           masked to window [p_t-16, p_t+16]
    align = softmax(score) * gauss(p_t)
    out   = tanh([align @ enc, h_t] @ W_c^T)

Only a 33-wide window of enc survives the mask, so windows are gathered
host-side and W_a / W_c[:, :H] are pushed through the 33-wide side:
    u  = W_a-transform of window   (uT[h, (n,j)]  = sum_k W_aT[k,h] enc_w[(n,j),k])
    s  = uT^T-partial scores       (score^T[j, q] = sum_h uT[h,j] h_t[q,h])
    softmax over j (33 rows) done j-major with a 4th-power renormalization
    trick (no partition-dim max needed; partition sums via ones-matmul)
    v  = W_c1-transform of window  (v[(n,j), h']  = sum_h enc_w[(n,j),h] W_c1T[h,h'])
    out = tanh(h_t @ W_c2T + align^T.T @ v)

All matmul operands are fp16 (11-bit mantissa == fp32r precision, same
1 row/cycle PE rate, half the DMA bytes / LDWEIGHTS time).  PSUM stays
fp32.  Softmax intermediates stay fp32/f32r (t spans up to e^83).

Schedule: the u phase runs kc-outer in two 4-PSUM-bank waves so the first
matmul only needs W_aT chunk 0; the whole v phase follows (W_c1 arrives
while u streams); then the 8-batch score/W_c2/softmax/ctx pipeline.  DMA
issues are spread across the sync/gpsimd/vector queues in arrival-priority
order so the PE never waits and keeps its p-state at the full 2.4 GHz.

Data parallel over batch: 8 batches per core x 8 cores.
"""

import numpy as np

import concourse.bass as bass
import concourse.bacc as bacc
import concourse.mybir as mybir
import concourse.tile as tile
from concourse.bass_utils import run_bass_kernel_spmd

# Problem constants (hardcoded per harness contract).
N, L, H, Q = 64, 258, 1024, 256
WINDOW = 16.0
DEV_POW = 128.0
NCORES = 8
B = N // NCORES  # batches per core
W = 33           # window width (positions that can survive the mask)
HC = H // 128    # h-chunks of 128 (PE contraction tiles)
F32 = mybir.dt.float32
F32R = mybir.dt.float32r
F16 = mybir.dt.float16
AF = mybir.ActivationFunctionType

# exp is computed as t = exp(s/4 + bias); bias = LOG_ALPHA keeps the
# column-sum T = sum_j t below fp32 max.  alpha cancels in w = t/T.
LOG_ALPHA = -4.8520302  # -7*ln(2)
MASK_BIAS = -10000.0    # exp(<= -9900) == 0 in fp32

# v-phase window groups: batches gi*3+off live at partitions off*W..off*W+32
GROUPS = [(0, 99), (99, 99), (198, 66)]


def build_nc() -> bass.Bass:
    nc = bacc.Bacc()
    enc_wT = nc.declare_dram_parameter("enc_wT", [H, B * W], F16, isOutput=False)
    dec_hT = nc.declare_dram_parameter("dec_hT", [H, B * Q], F16, isOutput=False)
    W_aT = nc.declare_dram_parameter("W_aT", [H, H], F16, isOutput=False)
    W_c1T = nc.declare_dram_parameter("W_c1T", [H, H], F16, isOutput=False)
    W_c2T = nc.declare_dram_parameter("W_c2T", [H, H], F16, isOutput=False)
    biasT = nc.declare_dram_parameter("biasT", [W, B], F32, isOutput=False)
    onesD = nc.declare_dram_parameter("onesD", [W, W], F32R, isOutput=False)
    gPackT = nc.declare_dram_parameter("gPackT", [3 * W, 3], F32, isOutput=False)
    out = nc.declare_dram_parameter("out", [B * Q, H], F16, isOutput=True)

    with tile.TileContext(nc) as tc:
        with (
            tc.tile_pool(name="const", bufs=1) as cpool,
            tc.tile_pool(name="wc1p", bufs=8) as wc1p,
            tc.tile_pool(name="vstp", bufs=6) as vstp,
            tc.tile_pool(name="dec", bufs=3) as dec_pool,
            tc.tile_pool(name="sm", bufs=4) as sm_pool,
            tc.tile_pool(name="outp", bufs=2) as out_pool,
            tc.tile_pool(name="psA", bufs=2, space="PSUM") as psA,
            tc.tile_pool(name="psB", bufs=6, space="PSUM") as psB,
        ):
            # ---------------- resident tensors ----------------
            enc_sb = cpool.tile([128, HC, B * W], F16)
            WaT_sb = cpool.tile([128, HC, H], F16)
            Wc2T_sb = cpool.tile([128, HC, H], F16)
            bias_sb = cpool.tile([W, B], F32)
            gpack_sb = cpool.tile([3 * W, 3], F32)
            ones_sb = cpool.tile([W, W], F32R)
            uT_sb = cpool.tile([128, HC, B * W], F16)
            # zero-padded align stacks: ctx contracts over a whole 99-row v
            # group; batch n's 33 softmax rows sit at partition (n%3)*W and
            # the zero rows cancel the other batches' windows exactly.
            t3_sb = [cpool.tile([3 * W, Q], F16, name=f"t3_{o}") for o in range(3)]

            enc_r = enc_wT[:, :].rearrange("(c p) m -> p c m", p=128)
            WaT_r = W_aT[:, :].rearrange("(c p) m -> p c m", p=128)
            Wc2_r = W_c2T[:, :].rearrange("(c p) m -> p c m", p=128)
            Wc1_r = W_c1T[:, :].rearrange("(cp p) m -> p cp m", p=128)
            dec_r = dec_hT[:, :].rearrange("(c p) (n q) -> p c n q", p=128, q=Q)

            # ---------------- DMA kickoff (issue order == priority) -------
            # u-phase operands stream on 3 queues in kc order; the first
            # matmul needs only (WaT kc0, enc kc0).
            nc.sync.dma_start(out=enc_sb[:, 0, :], in_=enc_r[:, 0, :])
            nc.gpsimd.dma_start(out=bias_sb, in_=biasT[:, :])
            nc.gpsimd.dma_start(out=ones_sb, in_=onesD[:, :])
            nc.gpsimd.dma_start(out=gpack_sb, in_=gPackT[:, :])
            for o in range(3):
                nc.gpsimd.memset(t3_sb[o][:, :], 0.0)
            qrr = [nc.sync, nc.gpsimd, nc.scalar]
            for kc in range(HC):
                qrr[kc % 3].dma_start(out=WaT_sb[:, kc, :], in_=WaT_r[:, kc, :])
                if kc > 0:
                    qrr[(kc + 1) % 3].dma_start(
                        out=enc_sb[:, kc, :], in_=enc_r[:, kc, :]
                    )

            # W_c1 tiles next (v phase starts ~16.5us), nt0 before nt1.
            wc1_tiles = {}
            for nt in range(2):
                for kcp in range(4):
                    wt = wc1p.tile([128, 2, 512], F16, tag="wc1", name=f"wc1_{nt}_{kcp}")
                    eng = nc.sync if kcp % 2 == 0 else nc.gpsimd
                    eng.dma_start(
                        out=wt,
                        in_=Wc1_r[:, 2 * kcp:2 * kcp + 2, nt * 512:(nt + 1) * 512],
                    )
                    wc1_tiles[(nt, kcp)] = wt

            # dec batches 0-1 and W_c2 (first needed ~27us) on the scalar
            # queue behind its WaT share (scalar is idle until the first exp).
            dec_tiles = {}
            for n in range(2):
                dt_ = dec_pool.tile([128, HC, Q], F16, tag="dec", name=f"dec{n}")
                nc.scalar.dma_start(out=dt_, in_=dec_r[:, :, n, :])
                dec_tiles[n] = dt_
            nc.scalar.dma_start(out=Wc2T_sb[:, :, 0:512], in_=Wc2_r[:, :, 0:512])
            nc.scalar.dma_start(out=Wc2T_sb[:, :, 512:1024], in_=Wc2_r[:, :, 512:1024])

            # ---------------- u phase: uT[h, (n,j)], kc-outer waves -------
            for wv in range(2):
                hcs = range(4 * wv, 4 * wv + 4)
                pu = {}
                for kc in range(HC):
                    for hc in hcs:
                        if kc == 0:
                            pu[hc] = psB.tile([128, B * W], F32, tag="B", name=f"pu{hc}")
                        nc.tensor.matmul(
                            pu[hc],
                            lhsT=WaT_sb[:, kc, hc * 128:(hc + 1) * 128],
                            rhs=enc_sb[:, kc, :],
                            start=(kc == 0),
                            stop=(kc == HC - 1),
                        )
                for i, hc in enumerate(hcs):
                    if i % 2 == 0:
                        nc.vector.tensor_copy(out=uT_sb[:, hc, :], in_=pu[hc])
                    else:
                        nc.scalar.copy(out=uT_sb[:, hc, :], in_=pu[hc])

            # ---------------- v phase: v[(n,j), h'] = enc_w @ W_c1T -------
            # vst tiles stay resident; ctx reads batch windows straight out
            # of them at partition offset (n%3)*W.
            vst_tiles = {}
            for nt in range(2):
                for gi in range(3):
                    g0, glen = GROUPS[gi]
                    pv = psB.tile([128, 512], F32, tag="B", name=f"pv{nt}_{gi}")
                    for kcp in range(4):
                        for j in range(2):
                            kc = 2 * kcp + j
                            nc.tensor.matmul(
                                pv[:glen, :],
                                lhsT=enc_sb[:, kc, g0:g0 + glen],
                                rhs=wc1_tiles[(nt, kcp)][:, j, :],
                                start=(kc == 0),
                                stop=(kc == HC - 1),
                            )
                    vst = vstp.tile([128, 512], F16, tag="vst", name=f"vst{nt}_{gi}")
                    # evacuate + fold the gaussian in one op
                    if (nt + gi) % 2 == 0:
                        nc.vector.tensor_scalar_mul(
                            vst[:glen, :], pv[:glen, :], gpack_sb[:glen, gi:gi + 1]
                        )
                    else:
                        nc.scalar.activation(
                            out=vst[:glen, :], in_=pv[:glen, :], func=AF.Copy,
                            scale=gpack_sb[:glen, gi:gi + 1],
                        )
                    vst_tiles[(nt, gi)] = vst

            # ---------------- per-batch pipeline ----------------
            prev = None  # (n, pos, o_sb) awaiting tanh + store

            def flush_prev(split_store=False):
                nonlocal prev
                if prev is None:
                    return
                pn, ppos, po_sb = prev
                for qt in range(2):
                    for ht in range(2):
                        nc.scalar.activation(
                            out=po_sb[:, qt, ht * 512:(ht + 1) * 512],
                            in_=ppos[(qt, ht)], func=AF.Tanh,
                        )
                dst = out[pn * Q:(pn + 1) * Q, :].rearrange("(qt p) h -> p qt h", p=128)
                if split_store:
                    nc.sync.dma_start(out=dst[:, 0, :], in_=po_sb[:, 0, :])
                    nc.gpsimd.dma_start(out=dst[:, 1, :], in_=po_sb[:, 1, :])
                else:
                    eng = nc.sync if pn % 2 == 0 else nc.gpsimd
                    eng.dma_start(out=dst, in_=po_sb)
                prev = None

            state = {}
            scored = {}

            def score_part(n):
                if n in dec_tiles:
                    dec_sb = dec_tiles.pop(n)
                else:
                    dec_sb = dec_pool.tile([128, HC, Q], F16, tag="dec", name=f"dec{n}")
                    nc.scalar.dma_start(out=dec_sb, in_=dec_r[:, :, n, :])
                # prefetch dec two batches ahead
                if n + 2 < B and n + 2 not in dec_tiles:
                    nxt = dec_pool.tile([128, HC, Q], F16, tag="dec", name=f"dec{n+2}")
                    nc.scalar.dma_start(out=nxt, in_=dec_r[:, :, n + 2, :])
                    dec_tiles[n + 2] = nxt

                ps = psA.tile([W, Q], F32, tag="A", name=f"ps{n}")
                for hc in range(HC):
                    nc.tensor.matmul(
                        ps,
                        lhsT=uT_sb[:, hc, n * W:(n + 1) * W],
                        rhs=dec_sb[:, hc, :],
                        start=(hc == 0),
                        stop=(hc == HC - 1),
                    )
                # softmax over 33 partitions via 4th-power renormalization:
                # t=exp(s/4+b); T=colsum t; t=(t/T)^4; Z=colsum t; t/=Z.
                t = sm_pool.tile([W, Q], F32R, tag="t", name=f"t{n}")
                nc.scalar.activation(
                    out=t, in_=ps, func=AF.Exp, bias=bias_sb[:, n:n + 1], scale=0.25
                )
                scored[n] = (dec_sb, t)

            def batch_pre(n):
                if n not in scored:
                    score_part(n)
                dec_sb, t = scored.pop(n)
                flush_prev()
                o_sb = out_pool.tile([128, 2, H], F16, tag="o", name=f"o{n}")
                pos = {}

                def dec_group(qt, ht):
                    po = psB.tile([128, 512], F32, tag="B", name=f"po{n}_{qt}_{ht}")
                    pos[(qt, ht)] = po
                    for hc in range(HC):
                        nc.tensor.matmul(
                            po,
                            lhsT=dec_sb[:, hc, qt * 128:(qt + 1) * 128],
                            rhs=Wc2T_sb[:, hc, ht * 512:(ht + 1) * 512],
                            start=(hc == 0),
                            stop=False,
                        )

                dec_group(0, 0)
                pT = psA.tile([W, Q], F32, tag="A", name=f"pT{n}")
                nc.tensor.matmul(pT, lhsT=ones_sb[:], rhs=t[:], start=True, stop=True)
                rT = sm_pool.tile([W, Q], F32, tag="rT", name=f"rT{n}")
                nc.vector.reciprocal_approx_fast(out=rT, in_=pT)
                nc.vector.tensor_mul(t, t, rT)
                nc.vector.tensor_mul(t, t, t)
                nc.vector.tensor_mul(t, t, t)
                dec_group(0, 1)
                dec_group(1, 0)
                pZ = psA.tile([W, Q], F32, tag="A", name=f"pZ{n}")
                nc.tensor.matmul(pZ, lhsT=ones_sb[:], rhs=t[:], start=True, stop=True)
                rZ = sm_pool.tile([W, Q], F32, tag="rZ", name=f"rZ{n}")
                nc.vector.reciprocal_approx_fast(out=rZ, in_=pZ)
                t16 = sm_pool.tile([W, Q], F16, tag="t16", name=f"t16_{n}")
                nc.vector.tensor_mul(t16, t, rZ)
                off = n % 3
                eng = nc.sync if n % 2 == 0 else nc.gpsimd
                eng.dma_start(out=t3_sb[off][off * W:(off + 1) * W, :], in_=t16[:, :])
                dec_group(1, 1)
                state[n] = (pos, o_sb)

            def batch_ctx(n):
                pos, o_sb = state.pop(n)
                last = n == B - 1
                gi, off = divmod(n, 3)
                glen = GROUPS[gi][1]
                dst = out[n * Q:(n + 1) * Q, :].rearrange("(qt p) h -> p qt h", p=128)
                for qt in range(2):
                    for ht in range(2):
                        nc.tensor.matmul(
                            pos[(qt, ht)],
                            lhsT=t3_sb[off][0:glen, qt * 128:(qt + 1) * 128],
                            rhs=vst_tiles[(ht, gi)][0:glen, :],
                            start=False,
                            stop=True,
                        )
                        if last:
                            # drain the tail in fine grain: tanh + store each
                            # [128, 256] chunk as soon as it closes
                            for hh in range(2):
                                c0 = ht * 512 + hh * 256
                                nc.scalar.activation(
                                    out=o_sb[:, qt, c0:c0 + 256],
                                    in_=pos[(qt, ht)][:, hh * 256:(hh + 1) * 256],
                                    func=AF.Tanh,
                                )
                                eng = (nc.sync, nc.gpsimd, nc.scalar, nc.sync)[ht * 2 + hh]
                                eng.dma_start(
                                    out=dst[:, qt, c0:c0 + 256],
                                    in_=o_sb[:, qt, c0:c0 + 256],
                                )
                nonlocal prev
                if not last:
                    prev = (n, pos, o_sb)

            for n in range(B):
                batch_pre(n)
                batch_ctx(n)
            flush_prev(split_store=True)
    nc.compile()
    return nc


def prepare_in_maps(inputs: dict) -> list[dict]:
    enc = np.asarray(inputs["encoder_outputs"], dtype=np.float32)
    dec = np.asarray(inputs["decoder_h_t"], dtype=np.float32)
    src_len = np.asarray(inputs["src_len"], dtype=np.int32)
    p_t = np.asarray(inputs["p_t"], dtype=np.float32)
    W_a = np.asarray(inputs["W_a"], dtype=np.float32)
    W_c = np.asarray(inputs["W_c"], dtype=np.float32)

    # Window bounds, computed with the same fp32 ops as the reference.
    attn_start = np.maximum(p_t - np.float32(WINDOW), np.float32(0.0))
    attn_end = np.minimum(p_t + np.float32(WINDOW), src_len.astype(np.float32))
    s = np.ceil(attn_start).astype(np.int64)
    s = np.minimum(s, L - W)  # keep the 33-slice in bounds
    idx = s[:, None] + np.arange(W)[None, :]
    idxf = idx.astype(np.float32)
    mask = (idxf < attn_start[:, None]) | (idxf > attn_end[:, None])
    bias = np.where(mask, np.float32(MASK_BIAS), np.float32(LOG_ALPHA)).astype(np.float32)
    g = np.exp(-((idxf - p_t[:, None]) ** 2) / np.float32(DEV_POW)).astype(np.float32)

    enc_w = enc[np.arange(N)[:, None], idx, :].astype(np.float16)  # [N, W, H]
    dec = dec.astype(np.float16)
    W_aT = W_a.T.astype(np.float16)
    W_c1T = W_c[:, :H].T.astype(np.float16)
    W_c2T = W_c[:, H:].T.astype(np.float16)

    in_maps = []
    for c in range(NCORES):
        bs = slice(c * B, (c + 1) * B)
        gc = g[bs]  # [B, W]
        gpack = np.zeros((3 * W, 3), dtype=np.float32)
        for n in range(B):
            gi, off = divmod(n, 3)
            gpack[off * W:(off + 1) * W, gi] = gc[n]
        in_maps.append({
            "enc_wT": np.ascontiguousarray(enc_w[bs].transpose(2, 0, 1).reshape(H, B * W)),
            "dec_hT": np.ascontiguousarray(dec[bs].transpose(2, 0, 1).reshape(H, B * Q)),
            "W_aT": W_aT,
            "W_c1T": W_c1T,
            "W_c2T": W_c2T,
            "biasT": np.ascontiguousarray(bias[bs].T),
            "onesD": np.ones((W, W), dtype=np.float32),
            "gPackT": gpack,
        })
    return in_maps


_NC = None


def get_nc() -> bass.Bass:
    global _NC
    if _NC is None:
        _NC = build_nc()
    return _NC


def kernel(**inputs) -> np.ndarray:
    nc = get_nc()
    in_maps = prepare_in_maps(inputs)
    res = run_bass_kernel_spmd(nc, in_maps, list(range(NCORES)))
    outs = [np.asarray(res.results[c]["out"], dtype=np.float32).reshape(B, Q, H)
            for c in range(NCORES)]
    return np.concatenate(outs, axis=0)


# revision 21
# speedup vs baseline: 1.0417x; 1.0270x over previous
"""Trainium2 Bass kernel for Luong local-p sparse attention.

Math (per batch n, full shapes N=64, L=258, H=1024, Q=256):
    score = (h_t @ W_a) @ enc^T           masked to window [p_t-16, p_t+16]
    align = softmax(score) * gauss(p_t)
    out   = tanh([align @ enc, h_t] @ W_c^T)

Only a 33-wide window of enc survives the mask, so windows are gathered
host-side and W_a / W_c[:, :H] are pushed through the 33-wide side:
    u  = W_a-transform of window   (uT[h, (n,j)]  = sum_k W_aT[k,h] enc_w[(n,j),k])
    s  = uT^T-partial scores       (score^T[j, q] = sum_h uT[h,j] h_t[q,h])
    softmax over j (33 rows) done j-major with a 4th-power renormalization
    trick (no partition-dim max needed; partition sums via ones-matmul)
    v  = W_c1-transform of window  (v[(n,j), h']  = sum_h enc_w[(n,j),h] W_c1T[h,h'])
    out = tanh(h_t @ W_c2T + align-stack @ v)

All matmuls run float32r (fp32_mode=HIGH streams 1 row / 2.4GHz-cycle; fp16
and bf16 matmuls only stream at 2.0GHz, measured).  The big streaming
inputs (W_aT+enc packed, dec) travel the wire as fp16 (same 11-bit mantissa
the PE keeps in f32r mode) and are upcast on-chip, halving load DMA.  The
u phase runs kc-outer over all 8 PSUM banks so the PE starts as soon as
the first packed chunk lands; the v phase follows immediately (W_c1
arrives while u streams); then the 8-batch score/W_c2/softmax/ctx
pipeline.  ctx contracts over a whole 99-row v group with a zero-padded
align stack (t3), so the v output never needs partition-shuffle DMAs.
The last batch is restructured: its softmax runs during batch 6, ctx
accumulates first, and each W_c2 quarter drains tanh+store as it closes.

Data parallel over batch: 8 batches per core x 8 cores.  Output is
stored fp16 (tanh output in [-1,1]; host upcasts).
"""

import numpy as np

import concourse.bass as bass
import concourse.bacc as bacc
import concourse.mybir as mybir
import concourse.tile as tile
from concourse.bass_utils import run_bass_kernel_spmd

# Problem constants (hardcoded per harness contract).
N, L, H, Q = 64, 258, 1024, 256
WINDOW = 16.0
DEV_POW = 128.0
NCORES = 8
B = N // NCORES  # batches per core
W = 33           # window width (positions that can survive the mask)
HC = H // 128    # h-chunks of 128 (PE contraction tiles)
PK = H + B * W   # packed [W_aT | enc_wT] row width
F32 = mybir.dt.float32
F32R = mybir.dt.float32r
F16 = mybir.dt.float16
AF = mybir.ActivationFunctionType

# exp is computed as t = exp(s/4 + bias); bias = LOG_ALPHA keeps the
# column-sum T = sum_j t below fp32 max.  alpha cancels in w = t/T.
LOG_ALPHA = -4.8520302  # -7*ln(2)
MASK_BIAS = -10000.0    # exp(<= -9900) == 0 in fp32

# v-phase window groups: batches gi*3+off live at partitions off*W..off*W+32
GROUPS = [(0, 99), (99, 99), (198, 66)]


def build_nc() -> bass.Bass:
    nc = bacc.Bacc()
    pk16 = nc.declare_dram_parameter("pk16", [H, PK], F16, isOutput=False)
    dec_hT = nc.declare_dram_parameter("dec_hT", [H, B * Q], F16, isOutput=False)
    W_c1T = nc.declare_dram_parameter("W_c1T", [H, H], F16, isOutput=False)
    W_c2T = nc.declare_dram_parameter("W_c2T", [H, H], F16, isOutput=False)
    constsD = nc.declare_dram_parameter("constsD", [3 * W, B + 3], F32, isOutput=False)
    onesD = nc.declare_dram_parameter("onesD", [W, W], F32R, isOutput=False)
    zerosD = nc.declare_dram_parameter("zerosD", [3 * W, Q], F32R, isOutput=False)
    out = nc.declare_dram_parameter("out", [B * Q, H], F16, isOutput=True)

    with tile.TileContext(nc) as tc:
        with (
            tc.tile_pool(name="const", bufs=1) as cpool,
            tc.tile_pool(name="stg", bufs=3) as stg_pool,
            tc.tile_pool(name="wstg", bufs=3) as wstg_pool,
            tc.tile_pool(name="wc1p", bufs=8) as wc1p,
            tc.tile_pool(name="vstp", bufs=6) as vstp,
            tc.tile_pool(name="dec16p", bufs=2) as dec16p,
            tc.tile_pool(name="dec", bufs=3) as dec_pool,
            tc.tile_pool(name="sm", bufs=4) as sm_pool,
            tc.tile_pool(name="outp", bufs=2) as out_pool,
            tc.tile_pool(name="psA", bufs=2, space="PSUM") as psA,
            tc.tile_pool(name="psB", bufs=6, space="PSUM") as psB,
        ):
            # ---------------- resident tensors ----------------
            pk32 = cpool.tile([128, HC, PK], F32R)   # [W_aT | enc] upcast
            Wc2T_sb = cpool.tile([128, HC, H], F32R)
            consts_sb = cpool.tile([3 * W, B + 3], F32)
            ones_sb = cpool.tile([W, W], F32R)
            uT_sb = cpool.tile([128, HC, B * W], F32R)
            # zero-padded align stacks: ctx contracts over a whole v group;
            # batch n's 33 softmax rows sit at partition (n%3)*W and the
            # zero rows cancel the other batches' windows exactly.
            t3_sb = [cpool.tile([3 * W, Q], F32R, name=f"t3_{o}") for o in range(3)]
            bias_ap = consts_sb[0:W, 0:B]

            pk_r = pk16[:, :].rearrange("(c p) m -> p c m", p=128)
            Wc2_r = W_c2T[:, :].rearrange("(c p) m -> p c m", p=128)
            Wc1_r = W_c1T[:, :].rearrange("(cp p) m -> p cp m", p=128)
            dec_r = dec_hT[:, :].rearrange("(c p) (n q) -> p c n q", p=128, q=Q)

            # ---------------- DMA kickoff (issue order == priority) -------
            # packed u-phase chunks round-robin on all three queues
            qrr = [nc.sync, nc.gpsimd, nc.scalar]
            pk_stage = {}
            for kc in range(HC):
                st = stg_pool.tile([128, PK], F16, tag="pk", name=f"pk{kc}")
                qrr[kc % 3].dma_start(out=st, in_=pk_r[:, kc, :])
                pk_stage[kc] = st
            nc.gpsimd.dma_start(out=consts_sb, in_=constsD[:, :])
            nc.gpsimd.dma_start(out=ones_sb, in_=onesD[:, :])
            for o in range(3):
                nc.gpsimd.dma_start(out=t3_sb[o][:, :], in_=zerosD[:, :])

            # dec batch 0 early on scalar queue (needed at first score)
            dec16_tiles = {}
            for n in range(2):
                d16 = dec16p.tile([128, HC, Q], F16, tag="d16", name=f"d16_{n}")
                nc.scalar.dma_start(out=d16, in_=dec_r[:, :, n, :])
                dec16_tiles[n] = d16

            # W_c1 tiles (v phase, ~18us): fp16 on the wire, upcast on the
            # scalar/vector engines as chunks land.
            wc1_tiles = {}
            for nt in range(2):
                for kcp in range(4):
                    st = wstg_pool.tile([128, 2, 512], F16, tag="w16",
                                        name=f"wc1s_{nt}_{kcp}")
                    eng = nc.sync if kcp % 2 == 0 else nc.gpsimd
                    eng.dma_start(
                        out=st,
                        in_=Wc1_r[:, 2 * kcp:2 * kcp + 2, nt * 512:(nt + 1) * 512],
                    )
                    wt = wc1p.tile([128, 2, 512], F32R, tag="wc1", name=f"wc1_{nt}_{kcp}")
                    if kcp % 2 == 0:
                        nc.scalar.copy(out=wt, in_=st)
                    else:
                        nc.vector.tensor_copy(out=wt, in_=st)
                    wc1_tiles[(nt, kcp)] = wt

            # W_c2 quarters (first dec_group ~27us), fp16 wire + upcast
            for qtr in range(4):
                st = wstg_pool.tile([128, HC, 256], F16, tag="w16", name=f"wc2s_{qtr}")
                eng = nc.sync if qtr % 2 == 0 else nc.gpsimd
                eng.dma_start(out=st, in_=Wc2_r[:, :, qtr * 256:(qtr + 1) * 256])
                dst = Wc2T_sb[:, :, qtr * 256:(qtr + 1) * 256]
                if qtr % 2 == 0:
                    nc.scalar.copy(out=dst, in_=st)
                else:
                    nc.vector.tensor_copy(out=dst, in_=st)

            # ---------------- u phase: uT[h, (n,j)], kc-outer -------------
            # upcast each packed chunk fp16 -> f32r as it lands, then stream
            # it through all 8 output chunks (uses all 8 PSUM banks).
            pu = {}
            for kc in range(HC):
                nc.vector.tensor_copy(out=pk32[:, kc, :], in_=pk_stage[kc])
                for hc in range(HC):
                    if kc == 0:
                        pool = psB if hc < 6 else psA
                        tag = "B" if hc < 6 else "A"
                        pu[hc] = pool.tile([128, B * W], F32, tag=tag, name=f"pu{hc}")
                    nc.tensor.matmul(
                        pu[hc],
                        lhsT=pk32[:, kc, hc * 128:(hc + 1) * 128],
                        rhs=pk32[:, kc, H:],
                        start=(kc == 0),
                        stop=(kc == HC - 1),
                    )
            for hc in range(HC):
                if hc % 2 == 0:
                    nc.vector.tensor_copy(out=uT_sb[:, hc, :], in_=pu[hc])
                else:
                    nc.scalar.copy(out=uT_sb[:, hc, :], in_=pu[hc])

            # ---------------- v phase: v[(n,j), h'] = enc_w @ W_c1T -------
            # vst tiles stay resident; ctx contracts over the whole group.
            vst_tiles = {}
            for nt in range(2):
                for gi in range(3):
                    g0, glen = GROUPS[gi]
                    pv = psB.tile([128, 512], F32, tag="B", name=f"pv{nt}_{gi}")
                    for kcp in range(4):
                        for j in range(2):
                            kc = 2 * kcp + j
                            nc.tensor.matmul(
                                pv[:glen, :],
                                lhsT=pk32[:, kc, H + g0:H + g0 + glen],
                                rhs=wc1_tiles[(nt, kcp)][:, j, :],
                                start=(kc == 0),
                                stop=(kc == HC - 1),
                            )
                    vst = vstp.tile([128, 512], F32R, tag="vst", name=f"vst{nt}_{gi}")
                    # evacuate + fold the gaussian in one op
                    gp = consts_sb[0:glen, B + gi:B + gi + 1]
                    if (nt + gi) % 2 == 0:
                        nc.vector.tensor_scalar_mul(vst[:glen, :], pv[:glen, :], gp)
                    else:
                        nc.scalar.activation(
                            out=vst[:glen, :], in_=pv[:glen, :], func=AF.Copy, scale=gp
                        )
                    vst_tiles[(nt, gi)] = vst

            # ---------------- per-batch pipeline ----------------
            prev = None  # (n, pos, o_sb) awaiting tanh + store

            def flush_prev():
                nonlocal prev
                if prev is None:
                    return
                pn, ppos, po_sb = prev
                for qt in range(2):
                    for ht in range(2):
                        nc.scalar.activation(
                            out=po_sb[:, qt, ht * 512:(ht + 1) * 512],
                            in_=ppos[(qt, ht)], func=AF.Tanh,
                        )
                dst = out[pn * Q:(pn + 1) * Q, :].rearrange("(qt p) h -> p qt h", p=128)
                eng = nc.sync if pn % 2 == 0 else nc.gpsimd
                eng.dma_start(out=dst, in_=po_sb)
                prev = None

            state = {}
            scored = {}

            def score_part(n):
                d16 = dec16_tiles.pop(n)
                # prefetch dec two batches ahead on the scalar queue
                if n + 2 < B and n + 2 not in dec16_tiles:
                    nxt = dec16p.tile([128, HC, Q], F16, tag="d16", name=f"d16_{n+2}")
                    nc.scalar.dma_start(out=nxt, in_=dec_r[:, :, n + 2, :])
                    dec16_tiles[n + 2] = nxt
                dec_sb = dec_pool.tile([128, HC, Q], F32R, tag="dec", name=f"dec{n}")
                nc.vector.tensor_copy(out=dec_sb, in_=d16)

                ps = psA.tile([W, Q], F32, tag="A", name=f"ps{n}")
                for hc in range(HC):
                    nc.tensor.matmul(
                        ps,
                        lhsT=uT_sb[:, hc, n * W:(n + 1) * W],
                        rhs=dec_sb[:, hc, :],
                        start=(hc == 0),
                        stop=(hc == HC - 1),
                    )
                # softmax over 33 partitions via 4th-power renormalization:
                # t=exp(s/4+b); T=colsum t; t=(t/T)^4; Z=colsum t; t/=Z.
                t = sm_pool.tile([W, Q], F32R, tag="t", name=f"t{n}")
                nc.scalar.activation(
                    out=t, in_=ps, func=AF.Exp, bias=bias_ap[:, n:n + 1], scale=0.25
                )
                scored[n] = (dec_sb, t)

            def softmax_T(n, t):
                pT = psA.tile([W, Q], F32, tag="A", name=f"pT{n}")
                nc.tensor.matmul(pT, lhsT=ones_sb[:], rhs=t[:], start=True, stop=True)
                rT = sm_pool.tile([W, Q], F32, tag="rT", name=f"rT{n}")
                nc.vector.reciprocal_approx_fast(out=rT, in_=pT)
                nc.vector.tensor_mul(t, t, rT)
                nc.vector.tensor_mul(t, t, t)
                nc.vector.tensor_mul(t, t, t)

            def softmax_Z(n, t):
                pZ = psA.tile([W, Q], F32, tag="A", name=f"pZ{n}")
                nc.tensor.matmul(pZ, lhsT=ones_sb[:], rhs=t[:], start=True, stop=True)
                rZ = sm_pool.tile([W, Q], F32, tag="rZ", name=f"rZ{n}")
                nc.vector.reciprocal_approx_fast(out=rZ, in_=pZ)
                tf = sm_pool.tile([W, Q], F32R, tag="tf", name=f"tf{n}")
                nc.vector.tensor_mul(tf, t, rZ)
                off = n % 3
                eng = nc.sync if n % 2 == 0 else nc.gpsimd
                eng.dma_start(out=t3_sb[off][off * W:(off + 1) * W, :], in_=tf[:, :])

            def dec_group(n, dec_sb, pos, qt, ht, start=True, stop=False):
                if start:
                    po = psB.tile([128, 512], F32, tag="B", name=f"po{n}_{qt}_{ht}")
                    pos[(qt, ht)] = po
                else:
                    po = pos[(qt, ht)]
                for hc in range(HC):
                    nc.tensor.matmul(
                        po,
                        lhsT=dec_sb[:, hc, qt * 128:(qt + 1) * 128],
                        rhs=Wc2T_sb[:, hc, ht * 512:(ht + 1) * 512],
                        start=start and (hc == 0),
                        stop=stop and (hc == HC - 1),
                    )

            def batch_pre(n):
                if n not in scored:
                    score_part(n)
                dec_sb, t = scored.pop(n)
                flush_prev()
                o_sb = out_pool.tile([128, 2, H], F16, tag="o", name=f"o{n}")
                pos = {}
                dec_group(n, dec_sb, pos, 0, 0)
                softmax_T(n, t)
                dec_group(n, dec_sb, pos, 0, 1)
                dec_group(n, dec_sb, pos, 1, 0)
                softmax_Z(n, t)
                dec_group(n, dec_sb, pos, 1, 1)
                state[n] = (pos, o_sb)

            def batch_ctx(n):
                pos, o_sb = state.pop(n)
                gi, off = divmod(n, 3)
                glen = GROUPS[gi][1]
                for qt in range(2):
                    for ht in range(2):
                        nc.tensor.matmul(
                            pos[(qt, ht)],
                            lhsT=t3_sb[off][0:glen, qt * 128:(qt + 1) * 128],
                            rhs=vst_tiles[(ht, gi)][0:glen, :],
                            start=False,
                            stop=True,
                        )
                nonlocal prev
                prev = (n, pos, o_sb)

            def batch_last(n):
                # softmax for n already ran during batch n-1; ctx accumulates
                # first, then each W_c2 quarter closes, tanhs and stores.
                dec_sb, _ = scored.pop(n)
                flush_prev()
                o_sb = out_pool.tile([128, 2, H], F16, tag="o", name=f"o{n}")
                gi, off = divmod(n, 3)
                glen = GROUPS[gi][1]
                dst = out[n * Q:(n + 1) * Q, :].rearrange("(qt p) h -> p qt h", p=128)
                pos = {}
                for qt in range(2):
                    for ht in range(2):
                        po = psB.tile([128, 512], F32, tag="B", name=f"po{n}_{qt}_{ht}")
                        pos[(qt, ht)] = po
                        nc.tensor.matmul(
                            po,
                            lhsT=t3_sb[off][0:glen, qt * 128:(qt + 1) * 128],
                            rhs=vst_tiles[(ht, gi)][0:glen, :],
                            start=True,
                            stop=False,
                        )
                for qt in range(2):
                    for ht in range(2):
                        dec_group(n, dec_sb, pos, qt, ht, start=False, stop=True)
                        nc.scalar.activation(
                            out=o_sb[:, qt, ht * 512:(ht + 1) * 512],
                            in_=pos[(qt, ht)], func=AF.Tanh,
                        )
                        eng = nc.sync if (qt + ht) % 2 == 0 else nc.gpsimd
                        eng.dma_start(
                            out=dst[:, qt, ht * 512:(ht + 1) * 512],
                            in_=o_sb[:, qt, ht * 512:(ht + 1) * 512],
                        )

            for n in range(B - 1):
                batch_pre(n)
                if n == B - 2:
                    # emit the last batch's score+softmax here so its t3 is
                    # ready before batch_last's ctx-first accumulation
                    score_part(B - 1)
                    _, t_last = scored[B - 1]
                    softmax_T(B - 1, t_last)
                    softmax_Z(B - 1, t_last)
                batch_ctx(n)
            batch_last(B - 1)
    nc.compile()
    return nc


def prepare_in_maps(inputs: dict) -> list[dict]:
    enc = np.asarray(inputs["encoder_outputs"], dtype=np.float32)
    dec = np.asarray(inputs["decoder_h_t"], dtype=np.float32)
    src_len = np.asarray(inputs["src_len"], dtype=np.int32)
    p_t = np.asarray(inputs["p_t"], dtype=np.float32)
    W_a = np.asarray(inputs["W_a"], dtype=np.float32)
    W_c = np.asarray(inputs["W_c"], dtype=np.float32)

    # Window bounds, computed with the same fp32 ops as the reference.
    attn_start = np.maximum(p_t - np.float32(WINDOW), np.float32(0.0))
    attn_end = np.minimum(p_t + np.float32(WINDOW), src_len.astype(np.float32))
    s = np.ceil(attn_start).astype(np.int64)
    s = np.minimum(s, L - W)  # keep the 33-slice in bounds
    idx = s[:, None] + np.arange(W)[None, :]
    idxf = idx.astype(np.float32)
    mask = (idxf < attn_start[:, None]) | (idxf > attn_end[:, None])
    bias = np.where(mask, np.float32(MASK_BIAS), np.float32(LOG_ALPHA)).astype(np.float32)
    g = np.exp(-((idxf - p_t[:, None]) ** 2) / np.float32(DEV_POW)).astype(np.float32)

    enc_w = enc[np.arange(N)[:, None], idx, :].astype(np.float16)  # [N, W, H]
    dec = dec.astype(np.float16)
    W_aT16 = W_a.T.astype(np.float16)
    W_c1T = W_c[:, :H].T.astype(np.float16)
    W_c2T = W_c[:, H:].T.astype(np.float16)

    in_maps = []
    for c in range(NCORES):
        bs = slice(c * B, (c + 1) * B)
        enc_wT = enc_w[bs].transpose(2, 0, 1).reshape(H, B * W)  # [H, B*W]
        pk = np.concatenate([W_aT16, enc_wT], axis=1)            # [H, H+B*W]
        gc = g[bs]  # [B, W]
        consts = np.zeros((3 * W, B + 3), dtype=np.float32)
        consts[0:W, 0:B] = bias[bs].T
        for n in range(B):
            gi, off = divmod(n, 3)
            consts[off * W:(off + 1) * W, B + gi] = gc[n]
        in_maps.append({
            "pk16": np.ascontiguousarray(pk),
            "dec_hT": np.ascontiguousarray(dec[bs].transpose(2, 0, 1).reshape(H, B * Q)),
            "W_c1T": W_c1T,
            "W_c2T": W_c2T,
            "constsD": consts,
            "onesD": np.ones((W, W), dtype=np.float32),
            "zerosD": np.zeros((3 * W, Q), dtype=np.float32),
        })
    return in_maps


_NC = None


def get_nc() -> bass.Bass:
    global _NC
    if _NC is None:
        _NC = build_nc()
    return _NC


def kernel(**inputs) -> np.ndarray:
    nc = get_nc()
    in_maps = prepare_in_maps(inputs)
    res = run_bass_kernel_spmd(nc, in_maps, list(range(NCORES)))
    outs = [np.asarray(res.results[c]["out"], dtype=np.float32).reshape(B, Q, H)
            for c in range(NCORES)]
    return np.concatenate(outs, axis=0)
